# revision 6
# baseline (speedup 1.0000x reference)
"""Grouped GEMM (MoE expert-parallel) on 8 TRN2 NeuronCores.

Strategy: expert-parallel — core e computes Y_e = X_e @ W_e^T for its expert's
contiguous token group.  Per core: [2048, 1024] @ [1024, 2048] -> [2048, 2048]
fp32.  Host side transposes the operands so the contraction dim (K=1024) lands
on SBUF partitions with fully contiguous DMA loads, runs the SPMD Bass kernel,
and scatters the per-expert results back into the full [16384, 2048] output.

The matmuls use the float32r PE path (1 cycle/row vs 4 for strict fp32).
"""

import numpy as np

import concourse.bass as bass  # noqa: F401  (kept for interactive debugging)
import concourse.mybir as mybir
import concourse.tile as tile
from concourse import bacc
from concourse.bass_utils import run_bass_kernel_spmd

NUM_CORES = 8
IN_F = 1024            # K (contraction)
OUT_F = 2048           # N (out features per expert)
CAP = 2048             # token capacity per core (= expected group size)
P = 128
KT = IN_F // P         # 8 k-subtiles
MT = CAP // P          # 16 m-tiles of tokens
NFREE = 512            # moving-operand free dim (one fp32 PSUM bank)
NT = OUT_F // NFREE    # 4 n-tiles

MM_DT = mybir.dt.float32r  # fast fp32 matmul path on the PE


def _build(repeat: int = 1):
    """Build the per-core Bass program: y[CAP, OUT_F] = xt.T @ wt.

    xt: [IN_F, CAP]   (X_e^T — K-major so K lands on partitions)
    wt: [IN_F, OUT_F] (W_e^T)
    """
    nc = bacc.Bacc(None, target_bir_lowering=False, debug=False)
    xt = nc.dram_tensor("xt", [IN_F, CAP], MM_DT, kind="ExternalInput")
    wt = nc.dram_tensor("wt", [IN_F, OUT_F], MM_DT, kind="ExternalInput")
    y = nc.dram_tensor("y", [CAP, OUT_F], mybir.dt.float32, kind="ExternalOutput")

    xtr = xt.rearrange("(o p) m -> p o m", p=P)   # [128, KT, CAP]
    wtr = wt.rearrange("(o p) n -> p o n", p=P)   # [128, KT, OUT_F]
    yr = y.rearrange("(mt p) n -> p mt n", p=P)   # [128, MT, OUT_F]

    with tile.TileContext(nc) as tc:
        with (
            tc.tile_pool(name="xt_pool", bufs=3) as xt_pool,
            tc.tile_pool(name="wt_pool", bufs=NT) as wt_pool,
            tc.tile_pool(name="out_pool", bufs=3) as out_pool,
            tc.tile_pool(name="psum", bufs=8, space="PSUM") as psum_pool,
        ):
            for _ in range(repeat):
                # wt tiles: one [128, KT, NFREE] tile per n-block, loaded on
                # first use (m==0), resident for all m.
                wt_tiles: dict[int, object] = {}
                for m in range(MT):
                    # all k-subtiles of this m-block in one DMA (0.5 MB)
                    xt_t = xt_pool.tile([P, KT, P], MM_DT, tag="xt")
                    nc.sync.dma_start(xt_t[:], xtr[:, :, m * P:(m + 1) * P])
                    y_sb = out_pool.tile([P, OUT_F], mybir.dt.float32, tag="y")
                    psums = []
                    for n in range(NT):
                        if n not in wt_tiles:
                            wtt = wt_pool.tile([P, KT, NFREE], MM_DT, tag="wt")
                            nc.sync.dma_start(
                                wtt[:], wtr[:, :, n * NFREE:(n + 1) * NFREE]
                            )
                            wt_tiles[n] = wtt
                        psum_t = psum_pool.tile(
                            [P, NFREE], mybir.dt.float32,
                            name=f"psum_{m}_{n}", tag="psum",
                        )
                        psums.append(psum_t)
                    for o in range(KT):
                        for n in range(NT):
                            nc.tensor.matmul(
                                psums[n],
                                lhsT=xt_t[:, o, :],
                                rhs=wt_tiles[n][:, o, :],
                                start=(o == 0),
                                stop=(o == KT - 1),
                            )
                    for n in range(NT):
                        nc.vector.tensor_copy(
                            y_sb[:, n * NFREE:(n + 1) * NFREE], psums[n][:]
                        )
                    nc.sync.dma_start(yr[:, m, :], y_sb[:])
    nc.compile()
    return nc


_NC_CACHE: dict = {}


def _get_nc(repeat: int = 1):
    if repeat not in _NC_CACHE:
        _NC_CACHE[repeat] = _build(repeat)
    return _NC_CACHE[repeat]


def _chunk_in_map(x, w, off: int, size: int, expert: int):
    """Build the per-core input map for one (expert, token-chunk)."""
    xe = np.zeros((CAP, IN_F), np.float32)
    if size > 0:
        xe[:size] = x[off:off + size]
    return {
        "xt": np.ascontiguousarray(xe.T),
        "wt": np.ascontiguousarray(w[expert].T),
    }


def kernel(**inputs) -> np.ndarray:
    x = np.asarray(inputs["input_tokens"], dtype=np.float32)       # [T, K]
    w = np.asarray(inputs["weight_stack"], dtype=np.float32)       # [E, O, K]
    m_sizes = np.asarray(inputs["m_sizes"]).astype(np.int64)       # [E]
    m_offsets = np.asarray(inputs["m_offsets"]).astype(np.int64)   # [E]

    T = x.shape[0]
    E, O, K = w.shape
    assert K == IN_F and O == OUT_F and E == NUM_CORES

    # Split each expert's contiguous token group into chunks of <= CAP rows
    # (the deterministic setup gives exactly one CAP-sized chunk per expert).
    chunks = []  # (expert, src_off, size)
    for e in range(E):
        off, size = int(m_offsets[e]), int(m_sizes[e])
        off = max(0, min(off, T))
        size = max(0, min(size, T - off))
        pos = 0
        while pos < size:
            c = min(CAP, size - pos)
            chunks.append((e, off + pos, c))
            pos += c

    out = np.zeros((T, O), dtype=np.float32)
    nc = _get_nc(1)
    for batch_start in range(0, len(chunks), NUM_CORES):
        batch = chunks[batch_start:batch_start + NUM_CORES]
        in_maps = [_chunk_in_map(x, w, off, size, e) for (e, off, size) in batch]
        # SPMD needs a full complement of cores; pad with repeats of map 0.
        while len(in_maps) < NUM_CORES:
            in_maps.append(in_maps[0])
        res = run_bass_kernel_spmd(nc, in_maps, list(range(NUM_CORES)))
        for i, (e, off, size) in enumerate(batch):
            ye = res.results[i]["y"]  # [CAP, OUT_F]
            out[off:off + size] += ye[:size]
    return out


# revision 7
# speedup vs baseline: 1740.8290x; 1740.8290x over previous
"""Grouped GEMM (MoE expert-parallel) on 8 TRN2 NeuronCores.

Strategy: expert-parallel — core e computes Y_e = X_e @ W_e^T for its expert's
contiguous token group.  Per core: [2048, 1024] @ [1024, 2048] -> [2048, 2048]
fp32.  Host side transposes the operands so the contraction dim (K=1024) lands
on SBUF partitions with fully contiguous DMA loads, runs the SPMD Bass kernel,
and scatters the per-expert results back into the full [16384, 2048] output.

The matmuls use the float32r PE path (1 cycle/row vs 4 for strict fp32).
"""

import numpy as np

import concourse.bass as bass  # noqa: F401  (kept for interactive debugging)
import concourse.mybir as mybir
import concourse.tile as tile
from concourse import bacc
from concourse.bass_utils import run_bass_kernel_spmd

NUM_CORES = 8
IN_F = 1024            # K (contraction)
OUT_F = 2048           # N (out features per expert)
CAP = 2048             # token capacity per core (= expected group size)
P = 128
KT = IN_F // P         # 8 k-subtiles
MT = CAP // P          # 16 m-tiles of tokens
NFREE = 512            # moving-operand free dim (one fp32 PSUM bank)
NT = OUT_F // NFREE    # 4 n-tiles

MM_DT = mybir.dt.float32r  # fast fp32 matmul path on the PE


def _build(repeat: int = 1):
    """Build the per-core Bass program: y[CAP, OUT_F] = xt.T @ wt.

    xt: [IN_F, CAP]   (X_e^T — K-major so K lands on partitions)
    wt: [IN_F, OUT_F] (W_e^T)
    """
    nc = bacc.Bacc(None, target_bir_lowering=False, debug=False)
    xt = nc.dram_tensor("xt", [IN_F, CAP], MM_DT, kind="ExternalInput")
    wt = nc.dram_tensor("wt", [IN_F, OUT_F], MM_DT, kind="ExternalInput")
    y = nc.dram_tensor("y", [CAP, OUT_F], mybir.dt.float32, kind="ExternalOutput")

    xtr = xt.rearrange("(o p) m -> p o m", p=P)   # [128, KT, CAP]
    wtr = wt.rearrange("(o p) n -> p o n", p=P)   # [128, KT, OUT_F]
    yr = y.rearrange("(mt p) n -> p mt n", p=P)   # [128, MT, OUT_F]

    with tile.TileContext(nc) as tc:
        WS = 2                 # wt k-split: 2 DMAs per n-block (finer deps)
        KS = KT // WS
        with (
            tc.tile_pool(name="xt_pool", bufs=3) as xt_pool,
            tc.tile_pool(name="wt_pool", bufs=NT * WS) as wt_pool,
            tc.tile_pool(name="out_pool", bufs=3) as out_pool,
            tc.tile_pool(name="psum", bufs=8, space="PSUM") as psum_pool,
        ):
            for _ in range(repeat):
                # wt tiles: WS tiles of [128, KS, NFREE] per n-block, loaded
                # on first use (m==0), resident for all m.
                wt_tiles: dict[int, list] = {}
                for m in range(MT):
                    # all k-subtiles of this m-block in one DMA (0.5 MB)
                    xt_t = xt_pool.tile([P, KT, P], MM_DT, tag="xt")
                    nc.sync.dma_start(xt_t[:], xtr[:, :, m * P:(m + 1) * P])
                    y_sb = out_pool.tile([P, OUT_F], mybir.dt.float32, tag="y")
                    psums = []
                    for n in range(NT):
                        if n not in wt_tiles:
                            parts = []
                            for s in range(WS):
                                wtt = wt_pool.tile(
                                    [P, KS, NFREE], MM_DT, tag="wt",
                                    name=f"wt_{n}_{s}",
                                )
                                nc.sync.dma_start(
                                    wtt[:],
                                    wtr[:, s * KS:(s + 1) * KS,
                                        n * NFREE:(n + 1) * NFREE],
                                )
                                parts.append(wtt)
                            wt_tiles[n] = parts
                        psum_t = psum_pool.tile(
                            [P, NFREE], mybir.dt.float32,
                            name=f"psum_{m}_{n}", tag="psum",
                        )
                        psums.append(psum_t)
                    for o in range(KT):
                        for n in range(NT):
                            nc.tensor.matmul(
                                psums[n],
                                lhsT=xt_t[:, o, :],
                                rhs=wt_tiles[n][o // KS][:, o % KS, :],
                                start=(o == 0),
                                stop=(o == KT - 1),
                            )
                    for n in range(NT):
                        nc.vector.tensor_copy(
                            y_sb[:, n * NFREE:(n + 1) * NFREE], psums[n][:]
                        )
                    nc.sync.dma_start(yr[:, m, :], y_sb[:])
    nc.compile()
    return nc


_NC_CACHE: dict = {}


def _get_nc(repeat: int = 1):
    if repeat not in _NC_CACHE:
        _NC_CACHE[repeat] = _build(repeat)
    return _NC_CACHE[repeat]


def _chunk_in_map(x, w, off: int, size: int, expert: int):
    """Build the per-core input map for one (expert, token-chunk)."""
    xe = np.zeros((CAP, IN_F), np.float32)
    if size > 0:
        xe[:size] = x[off:off + size]
    return {
        "xt": np.ascontiguousarray(xe.T),
        "wt": np.ascontiguousarray(w[expert].T),
    }


def kernel(**inputs) -> np.ndarray:
    x = np.asarray(inputs["input_tokens"], dtype=np.float32)       # [T, K]
    w = np.asarray(inputs["weight_stack"], dtype=np.float32)       # [E, O, K]
    m_sizes = np.asarray(inputs["m_sizes"]).astype(np.int64)       # [E]
    m_offsets = np.asarray(inputs["m_offsets"]).astype(np.int64)   # [E]

    T = x.shape[0]
    E, O, K = w.shape
    assert K == IN_F and O == OUT_F and E == NUM_CORES

    # Split each expert's contiguous token group into chunks of <= CAP rows
    # (the deterministic setup gives exactly one CAP-sized chunk per expert).
    chunks = []  # (expert, src_off, size)
    for e in range(E):
        off, size = int(m_offsets[e]), int(m_sizes[e])
        off = max(0, min(off, T))
        size = max(0, min(size, T - off))
        pos = 0
        while pos < size:
            c = min(CAP, size - pos)
            chunks.append((e, off + pos, c))
            pos += c

    out = np.zeros((T, O), dtype=np.float32)
    nc = _get_nc(1)
    for batch_start in range(0, len(chunks), NUM_CORES):
        batch = chunks[batch_start:batch_start + NUM_CORES]
        in_maps = [_chunk_in_map(x, w, off, size, e) for (e, off, size) in batch]
        # SPMD needs a full complement of cores; pad with repeats of map 0.
        while len(in_maps) < NUM_CORES:
            in_maps.append(in_maps[0])
        res = run_bass_kernel_spmd(nc, in_maps, list(range(NUM_CORES)))
        for i, (e, off, size) in enumerate(batch):
            ye = res.results[i]["y"]  # [CAP, OUT_F]
            out[off:off + size] += ye[:size]
    return out
